# revision 6
# baseline (speedup 1.0000x reference)
"""Trainium2 Bass kernel for the ObjectEncoder problem.

Pipeline (data-parallel over bs_t=48 frames, 6 frames per core x 8 cores):
  host: build per-(frame,box) separable bilinear-interp matrices and fuse
        them into one dense kron matrix M_all[s=(y,x), (o,p,q)] per frame.
        roi_align is linear in the image, so
           roi_align(img) @ W1 == M_all.T @ (img @ W1)
        which shrinks the interp matmul from d=768 to k=384 columns.
  core: A   = img @ W1                      (256, 384)  per frame
        h^T = relu(A^T-by-chunks @ M_all)   (384, 2560) per frame
        o^T = h^T.T-contract @ W2           (768, 2560) -> max over 256 cells
        out = relu-fold(max) + box_mlp(boxes) + box_categories
All matmuls run as float32r (full-rate fp32 mode on the PE).
"""

import numpy as np
from contextlib import ExitStack

import concourse.bass as bass
import concourse.mybir as mybir
from concourse import tile
from concourse.bass_utils import run_bass_kernel_spmd

F32 = mybir.dt.float32
F32R = mybir.dt.float32r

N_CORES = 8
PATCH = 16
IN_DIM = 768
KDIM = 384
NB_FRAMES = 12
OBJ = 10
BS = 4
BS_T = BS * NB_FRAMES            # 48
FPC = BS_T // N_CORES            # 6 frames per core
CELLS = PATCH * PATCH            # 256
NCOLS = OBJ * CELLS              # 2560
NCHUNK = 512                     # psum-bank-sized moving chunks
NCH = NCOLS // NCHUNK            # 5
VIDEO_HW = 224.0


def _patch_tile_drain():
    """walrus in this container allows only 1 sync-wait on a CTRL (Drain)
    instruction; spread the Tile tail-drain waits across SP nops."""
    if getattr(tile.TileContext, "_drain_patched", False):
        return
    ScopedClock = tile.ScopedClock

    def _drain_and_barrier(self, tick_clock, wait_clock):
        probe = self.nc.sync.drain()
        wait_clock.add_sem_waits(
            probe.ins, ScopedClock({None: tick_clock.global_clock})
        )
        waits = list(probe.ins.sync_info.on_wait)
        probe.ins.sync_info.on_wait = waits[:1]
        for w in waits[1:]:
            nop = self.nc.sync.nop(nofuse=True)
            nop.ins.sync_info = mybir.SyncInfo(on_wait=[w], on_update=[])
        self.nc.all_engine_barrier()
        popped = self.nc._tile_sem_poison_stack.pop()
        assert popped is self._sem_poison
        self.nc.clear_and_free_semaphores(list(self.sems.allocated().values()))
        self.nc.all_engine_barrier()

    tile.TileContext._drain_and_barrier = _drain_and_barrier
    tile.TileContext._drain_patched = True


_WAIT_CAP = 1
_SPLIT_SKIP = set()


def _split_excess_waits(nc):
    """This container's walrus rejects >1 sync-wait on compute/CTRL
    instructions; move excess waits onto same-engine NoOps placed before."""
    ctr = [0]
    for fn in nc.m.functions:
        for bb in fn.blocks:
            new_list = []
            for inst in bb.instructions:
                si = getattr(inst, "sync_info", None)
                waits = list(si.on_wait) if si and si.on_wait else []
                if (
                    len(waits) > _WAIT_CAP
                    and type(inst).__name__ not in _SPLIT_SKIP
                ):
                    for w in waits[:-_WAIT_CAP]:
                        ctr[0] += 1
                        nop = mybir.InstNoOp(
                            name=f"WSPLIT-{ctr[0]}", ins=[], outs=[]
                        )
                        nop.engine = inst.engine
                        nop.sync_info = mybir.SyncInfo(
                            on_wait=[w], on_update=[]
                        )
                        new_list.append(nop)
                    si.on_wait = waits[-_WAIT_CAP:]
                new_list.append(inst)
            bb.instructions = new_list


def _axis_interp_np(coord, size):
    # mirrors reference._axis_interp (torchvision bilinear semantics)
    valid = ((coord >= -1.0) & (coord <= size)).astype(coord.dtype)
    c = np.maximum(coord, np.float32(0.0))
    i0 = np.minimum(np.floor(c).astype(np.int64), size - 1)
    i1 = np.minimum(i0 + 1, size - 1)
    l = np.clip(c - i0.astype(coord.dtype), 0.0, 1.0)
    w0 = (1.0 - l) * valid
    w1 = l * valid
    return i0, i1, w0, w1


def _build_interp_matrices(boxes_f):
    """boxes_f: (F, O, 4) fp32 -> M_all (F, 256, O*256) fp32 with
    M_all[f, y*16+x, o*256+p*16+q] = My[f,o,p,y] * Mx[f,o,q,x]."""
    F = boxes_f.shape[0]
    scale = np.float32(PATCH / VIDEO_HW)
    b = boxes_f.astype(np.float32) * scale - np.float32(0.5)
    x1, y1, x2, y2 = b[..., 0], b[..., 1], b[..., 2], b[..., 3]
    grid = (np.arange(PATCH, dtype=np.float32) + np.float32(0.5))
    ys = y1[..., None] + grid * ((y2 - y1) / PATCH)[..., None]   # (F,O,P)
    xs = x1[..., None] + grid * ((x2 - x1) / PATCH)[..., None]   # (F,O,P)

    def weight_mat(coord):
        i0, i1, w0, w1 = _axis_interp_np(coord, PATCH)
        M = np.zeros((F, OBJ, PATCH, PATCH), np.float32)
        fi, oi, pi = np.indices(i0.shape)
        np.add.at(M, (fi, oi, pi, i0), w0)
        np.add.at(M, (fi, oi, pi, i1), w1)
        return M

    My = weight_mat(ys)   # (F, O, P, 16) over y
    Mx = weight_mat(xs)   # (F, O, P, 16) over x
    M_all = np.einsum("fopy,foqx->fyxopq", My, Mx)
    return np.ascontiguousarray(M_all.reshape(F, CELLS, NCOLS))


_CACHE = {}


def _build_module():
    _patch_tile_drain()
    nc = bass.Bass()
    imgT = nc.declare_dram_parameter("imgT", [FPC, IN_DIM, CELLS], F32R, isOutput=False)
    mall = nc.declare_dram_parameter("mall", [FPC, CELLS, NCOLS], F32R, isOutput=False)
    w1 = nc.declare_dram_parameter("w1", [IN_DIM, KDIM], F32R, isOutput=False)
    w2 = nc.declare_dram_parameter("w2", [KDIM, IN_DIM], F32R, isOutput=False)
    c1 = nc.declare_dram_parameter("c1", [4, KDIM], F32R, isOutput=False)
    c2 = nc.declare_dram_parameter("c2", [KDIM, IN_DIM], F32R, isOutput=False)
    boxn = nc.declare_dram_parameter("boxn", [4, FPC * OBJ], F32R, isOutput=False)
    bcatT = nc.declare_dram_parameter("bcatT", [IN_DIM, FPC * OBJ], F32, isOutput=False)
    out = nc.declare_dram_parameter("out", [6, 128, FPC * OBJ], F32, isOutput=True)

    r = lambda ap: ap
    Relu = mybir.ActivationFunctionType.Relu
    NFO = FPC * OBJ  # 60

    tc = tile.TileContext(nc)
    with tc, ExitStack() as ctx:
        const = ctx.enter_context(tc.tile_pool(name="const", bufs=1))
        img_pool = ctx.enter_context(tc.tile_pool(name="img", bufs=2))
        mall_pool = ctx.enter_context(tc.tile_pool(name="mall", bufs=2))
        a_pool = ctx.enter_context(tc.tile_pool(name="a", bufs=2))
        h_pool = ctx.enter_context(tc.tile_pool(name="h", bufs=6))
        psA = ctx.enter_context(tc.tile_pool(name="psA", bufs=2, space="PSUM"))
        psH = ctx.enter_context(tc.tile_pool(name="psH", bufs=2, space="PSUM"))
        psO = ctx.enter_context(tc.tile_pool(name="psO", bufs=2, space="PSUM"))

        # ---- constants
        w1_t = const.tile([128, 6, KDIM], F32R)
        nc.sync.dma_start(out=w1_t[:], in_=w1.rearrange("(a p) k -> p a k", p=128))
        w2_t = const.tile([128, 3, IN_DIM], F32R)
        nc.sync.dma_start(out=w2_t[:], in_=w2.rearrange("(a p) d -> p a d", p=128))
        c2_t = const.tile([128, 3, IN_DIM], F32R)
        nc.sync.dma_start(out=c2_t[:], in_=c2.rearrange("(a p) d -> p a d", p=128))
        c1_t = const.tile([4, KDIM], F32R)
        nc.sync.dma_start(out=c1_t[:], in_=c1[:])
        boxn_t = const.tile([4, NFO], F32R)
        nc.sync.dma_start(out=boxn_t[:], in_=boxn[:])
        bcat_t = const.tile([128, 6, NFO], F32)
        nc.sync.dma_start(out=bcat_t[:], in_=bcatT.rearrange("(a p) n -> p a n", p=128))

        # ---- box MLP: bemb^T = relu(relu(boxn^T @ C1) @ C2)^T  (d-major)
        be_t = const.tile([128, 3, NFO], F32R)
        for m3 in range(3):
            pb = psA.tile([128, NFO], F32)
            nc.tensor.matmul(
                pb[:], lhsT=r(c1_t[:, bass.ts(m3, 128)]), rhs=r(boxn_t[:]),
                start=True, stop=True,
            )
            nc.scalar.activation(be_t[:, m3, :], pb[:], Relu)
        bemb_t = const.tile([128, 6, NFO], F32)
        for m6 in range(6):
            pb = psA.tile([128, NFO], F32)
            for k3 in range(3):
                nc.tensor.matmul(
                    pb[:], lhsT=r(c2_t[:, k3, bass.ts(m6, 128)]),
                    rhs=r(be_t[:, k3, :]), start=(k3 == 0), stop=(k3 == 2),
                )
            nc.scalar.activation(bemb_t[:, m6, :], pb[:], Relu)

        # ---- per-frame: A = img @ W1 ; h = relu(interp) ; o = h @ W2 ; max
        objmax_t = const.tile([128, 6, FPC, OBJ], F32)
        for f in range(FPC):
            imgT_t = img_pool.tile([128, 6, CELLS], F32R)
            nc.sync.dma_start(
                out=imgT_t[:], in_=imgT[f].rearrange("(a p) s -> p a s", p=128)
            )
            mall_t = mall_pool.tile([128, 2, NCOLS], F32R)
            nc.sync.dma_start(
                out=mall_t[:], in_=mall[f].rearrange("(a p) n -> p a n", p=128)
            )
            # A (s-major): psum[ms] = sum_kt imgT[kt, ms-chunk].T @ W1[kt]
            a_t = a_pool.tile([128, 2, KDIM], F32R)
            for ms in range(2):
                pa = psA.tile([128, KDIM], F32)
                for kt in range(6):
                    nc.tensor.matmul(
                        pa[:], lhsT=r(imgT_t[:, kt, bass.ts(ms, 128)]),
                        rhs=r(w1_t[:, kt, :]), start=(kt == 0), stop=(kt == 5),
                    )
                nc.scalar.copy(a_t[:, ms, :], pa[:])
            for nch in range(NCH):
                hs = []
                for m3 in range(3):
                    ph = psH.tile([128, NCHUNK], F32)
                    for st in range(2):
                        nc.tensor.matmul(
                            ph[:], lhsT=r(a_t[:, st, bass.ts(m3, 128)]),
                            rhs=r(mall_t[:, st, bass.ts(nch, NCHUNK)]),
                            start=(st == 0), stop=(st == 1),
                        )
                    h_t = h_pool.tile([128, NCHUNK], F32R)
                    nc.scalar.activation(h_t[:], ph[:], Relu)
                    hs.append(h_t)
                for m6 in range(6):
                    po = psO.tile([128, NCHUNK], F32)
                    for k3 in range(3):
                        nc.tensor.matmul(
                            po[:], lhsT=r(w2_t[:, k3, bass.ts(m6, 128)]),
                            rhs=r(hs[k3][:]), start=(k3 == 0), stop=(k3 == 2),
                        )
                    nc.vector.tensor_reduce(
                        out=objmax_t[:, m6, f, 2 * nch : 2 * nch + 2],
                        in_=po.rearrange("p (o q) -> p o q", q=CELLS),
                        axis=mybir.AxisListType.X,
                        op=mybir.AluOpType.max,
                    )

        # ---- combine: relu-fold(max) + box_emb + box_categories, d-major out
        out_t = const.tile([128, 6, NFO], F32)
        for m6 in range(6):
            nc.vector.scalar_tensor_tensor(
                out=out_t[:, m6, :],
                in0=objmax_t.rearrange("p a f o -> p a (f o)")[:, m6, :],
                scalar=0.0,
                in1=bemb_t[:, m6, :],
                op0=mybir.AluOpType.max,
                op1=mybir.AluOpType.add,
            )
            nc.vector.tensor_add(out_t[:, m6, :], out_t[:, m6, :], bcat_t[:, m6, :])
        nc.sync.dma_start(out=out.rearrange("a p n -> p a n"), in_=out_t[:])

    _split_excess_waits(nc)
    return nc


def get_module():
    if "nc" not in _CACHE:
        _CACHE["nc"] = _build_module()
    return _CACHE["nc"]


def make_in_maps(features, boxes, W1, W2, C1, C2, box_categories):
    features = np.asarray(features, np.float32)
    boxes = np.asarray(boxes, np.float32)
    boxes_f = boxes.reshape(BS_T, OBJ, 4)
    m_all = _build_interp_matrices(boxes_f)
    w1 = np.ascontiguousarray(np.asarray(W1, np.float32))
    w2 = np.ascontiguousarray(np.asarray(W2, np.float32))
    c1 = np.ascontiguousarray(np.asarray(C1, np.float32))
    c2 = np.ascontiguousarray(np.asarray(C2, np.float32))
    bcat = np.asarray(box_categories, np.float32)

    in_maps = []
    for c in range(N_CORES):
        fr = slice(c * FPC, (c + 1) * FPC)
        imgT = np.ascontiguousarray(
            features[1:, fr, :].transpose(1, 2, 0)  # (6, 768, 256)
        )
        mall_c = np.ascontiguousarray(m_all[fr])
        boxn = np.ascontiguousarray(
            (boxes_f[fr].reshape(FPC * OBJ, 4).T / np.float32(VIDEO_HW))
        )
        t_idx = (np.arange(c * FPC, (c + 1) * FPC)) % NB_FRAMES
        bcatT = np.ascontiguousarray(
            bcat[t_idx].reshape(FPC * OBJ, IN_DIM).T  # (768, 60)
        )
        in_maps.append(
            {
                "imgT": imgT,
                "mall": mall_c,
                "w1": w1,
                "w2": w2,
                "c1": c1,
                "c2": c2,
                "boxn": boxn,
                "bcatT": bcatT,
            }
        )
    return in_maps


def assemble_output(results, boxes):
    boxes_f = np.asarray(boxes, np.float32).reshape(BS_T, OBJ, 4)
    object_tokens = np.empty((OBJ, BS_T, IN_DIM), np.float32)
    for c in range(N_CORES):
        arr = results[c]["out"].reshape(IN_DIM, FPC, OBJ)  # (768, 6, 10)
        object_tokens[:, c * FPC : (c + 1) * FPC, :] = arr.transpose(2, 1, 0)
    attn_mask = boxes_f[:, :, 0] == -1.0
    return object_tokens, attn_mask


def kernel(features, boxes, W1, W2, C1, C2, box_categories):
    nc = get_module()
    in_maps = make_in_maps(features, boxes, W1, W2, C1, C2, box_categories)
    res = run_bass_kernel_spmd(nc, in_maps, list(range(N_CORES)))
    return assemble_output(res.results, boxes)


# revision 9
# speedup vs baseline: 88.4539x; 88.4539x over previous
"""Trainium2 Bass kernel for the ObjectEncoder problem.

Pipeline (data-parallel over bs_t=48 frames, 6 frames per core x 8 cores):
  host: build per-(frame,box) separable bilinear-interp matrices and fuse
        them into one dense kron matrix M_all[s=(y,x), (o,p,q)] per frame.
        roi_align is linear in the image, so
           roi_align(img) @ W1 == M_all.T @ (img @ W1)
        which shrinks the interp matmul from d=768 to k=384 columns.
  core: A   = img @ W1                      (256, 384)  per frame
        h^T = relu(A^T-by-chunks @ M_all)   (384, 2560) per frame
        o^T = h^T.T-contract @ W2           (768, 2560) -> max over 256 cells
        out = relu-fold(max) + box_mlp(boxes) + box_categories
All matmuls run as float32r (full-rate fp32 mode on the PE).
"""

import numpy as np
from contextlib import ExitStack

import concourse.bass as bass
import concourse.mybir as mybir
from concourse import tile
from concourse.bass_utils import run_bass_kernel_spmd

F32 = mybir.dt.float32
F32R = mybir.dt.float32r

N_CORES = 8
PATCH = 16
IN_DIM = 768
KDIM = 384
NB_FRAMES = 12
OBJ = 10
BS = 4
BS_T = BS * NB_FRAMES            # 48
FPC = BS_T // N_CORES            # 6 frames per core
CELLS = PATCH * PATCH            # 256
NCOLS = OBJ * CELLS              # 2560
NCHUNK = 512                     # psum-bank-sized moving chunks
NCH = NCOLS // NCHUNK            # 5
VIDEO_HW = 224.0


def _patch_tile_drain():
    """walrus in this container allows only 1 sync-wait on a CTRL (Drain)
    instruction; spread the Tile tail-drain waits across SP nops."""
    if getattr(tile.TileContext, "_drain_patched", False):
        return
    ScopedClock = tile.ScopedClock

    def _drain_and_barrier(self, tick_clock, wait_clock):
        probe = self.nc.sync.drain()
        wait_clock.add_sem_waits(
            probe.ins, ScopedClock({None: tick_clock.global_clock})
        )
        waits = list(probe.ins.sync_info.on_wait)
        probe.ins.sync_info.on_wait = waits[:1]
        for w in waits[1:]:
            nop = self.nc.sync.nop(nofuse=True)
            nop.ins.sync_info = mybir.SyncInfo(on_wait=[w], on_update=[])
        self.nc.all_engine_barrier()
        popped = self.nc._tile_sem_poison_stack.pop()
        assert popped is self._sem_poison
        self.nc.clear_and_free_semaphores(list(self.sems.allocated().values()))
        self.nc.all_engine_barrier()

    tile.TileContext._drain_and_barrier = _drain_and_barrier
    tile.TileContext._drain_patched = True


_WAIT_CAP = 1
_SPLIT_SKIP = set()


def _split_excess_waits(nc):
    """This container's walrus rejects >1 sync-wait on compute/CTRL
    instructions; move excess waits onto same-engine NoOps placed before."""
    ctr = [0]
    for fn in nc.m.functions:
        for bb in fn.blocks:
            new_list = []
            for inst in bb.instructions:
                si = getattr(inst, "sync_info", None)
                waits = list(si.on_wait) if si and si.on_wait else []
                if (
                    len(waits) > _WAIT_CAP
                    and type(inst).__name__ not in _SPLIT_SKIP
                ):
                    for w in waits[:-_WAIT_CAP]:
                        ctr[0] += 1
                        nop = mybir.InstNoOp(
                            name=f"WSPLIT-{ctr[0]}", ins=[], outs=[]
                        )
                        nop.engine = inst.engine
                        nop.sync_info = mybir.SyncInfo(
                            on_wait=[w], on_update=[]
                        )
                        new_list.append(nop)
                    si.on_wait = waits[-_WAIT_CAP:]
                new_list.append(inst)
            bb.instructions = new_list


def _axis_interp_np(coord, size):
    # mirrors reference._axis_interp (torchvision bilinear semantics)
    valid = ((coord >= -1.0) & (coord <= size)).astype(coord.dtype)
    c = np.maximum(coord, np.float32(0.0))
    i0 = np.minimum(np.floor(c).astype(np.int64), size - 1)
    i1 = np.minimum(i0 + 1, size - 1)
    l = np.clip(c - i0.astype(coord.dtype), 0.0, 1.0)
    w0 = (1.0 - l) * valid
    w1 = l * valid
    return i0, i1, w0, w1


def _build_interp_matrices(boxes_f):
    """boxes_f: (F, O, 4) fp32 -> M_all (F, 256, O*256) fp32 with
    M_all[f, y*16+x, o*256+p*16+q] = My[f,o,p,y] * Mx[f,o,q,x]."""
    F = boxes_f.shape[0]
    scale = np.float32(PATCH / VIDEO_HW)
    b = boxes_f.astype(np.float32) * scale - np.float32(0.5)
    x1, y1, x2, y2 = b[..., 0], b[..., 1], b[..., 2], b[..., 3]
    grid = (np.arange(PATCH, dtype=np.float32) + np.float32(0.5))
    ys = y1[..., None] + grid * ((y2 - y1) / PATCH)[..., None]   # (F,O,P)
    xs = x1[..., None] + grid * ((x2 - x1) / PATCH)[..., None]   # (F,O,P)

    def weight_mat(coord):
        i0, i1, w0, w1 = _axis_interp_np(coord, PATCH)
        M = np.zeros((F, OBJ, PATCH, PATCH), np.float32)
        fi, oi, pi = np.indices(i0.shape)
        np.add.at(M, (fi, oi, pi, i0), w0)
        np.add.at(M, (fi, oi, pi, i1), w1)
        return M

    My = weight_mat(ys)   # (F, O, P, 16) over y
    Mx = weight_mat(xs)   # (F, O, P, 16) over x
    M_all = np.einsum("fopy,foqx->fyxopq", My, Mx)
    return np.ascontiguousarray(M_all.reshape(F, CELLS, NCOLS))


_CACHE = {}


def _build_module(reps=1, io_lite=False):
    """reps>1 replicates the whole frame pipeline for wall-clock timing
    amplification; io_lite turns the big inputs into internal (uninitialized)
    DRAM so timing runs don't pay host->device transfer."""
    _patch_tile_drain()
    nc = bass.Bass()
    if io_lite:
        imgT = nc.dram_tensor("imgT_i", [FPC, IN_DIM, CELLS], F32R)
        mall = nc.dram_tensor("mall_i", [FPC, CELLS, NCOLS], F32R)
    else:
        imgT = nc.declare_dram_parameter(
            "imgT", [FPC, IN_DIM, CELLS], F32R, isOutput=False
        )
        mall = nc.declare_dram_parameter(
            "mall", [FPC, CELLS, NCOLS], F32R, isOutput=False
        )
    w1 = nc.declare_dram_parameter("w1", [IN_DIM, KDIM], F32R, isOutput=False)
    w2 = nc.declare_dram_parameter("w2", [KDIM, IN_DIM], F32R, isOutput=False)
    c1 = nc.declare_dram_parameter("c1", [4, KDIM], F32R, isOutput=False)
    c2 = nc.declare_dram_parameter("c2", [KDIM, IN_DIM], F32R, isOutput=False)
    boxn = nc.declare_dram_parameter("boxn", [4, FPC * OBJ], F32R, isOutput=False)
    bcatT = nc.declare_dram_parameter("bcatT", [IN_DIM, FPC * OBJ], F32, isOutput=False)
    out = nc.declare_dram_parameter("out", [6, 128, FPC * OBJ], F32, isOutput=True)

    r = lambda ap: ap
    Relu = mybir.ActivationFunctionType.Relu
    NFO = FPC * OBJ  # 60

    tc = tile.TileContext(nc)
    with tc, ExitStack() as ctx:
        const = ctx.enter_context(tc.tile_pool(name="const", bufs=1))
        img_pool = ctx.enter_context(tc.tile_pool(name="img", bufs=2))
        mall_pool = ctx.enter_context(tc.tile_pool(name="mall", bufs=2))
        a_pool = ctx.enter_context(tc.tile_pool(name="a", bufs=2))
        h_pool = ctx.enter_context(tc.tile_pool(name="h", bufs=6))
        psA = ctx.enter_context(tc.tile_pool(name="psA", bufs=2, space="PSUM"))
        psH = ctx.enter_context(tc.tile_pool(name="psH", bufs=2, space="PSUM"))
        psO = ctx.enter_context(tc.tile_pool(name="psO", bufs=2, space="PSUM"))

        # ---- constants
        w1_t = const.tile([128, 6, KDIM], F32R)
        nc.sync.dma_start(out=w1_t[:], in_=w1.rearrange("(a p) k -> p a k", p=128))
        w2_t = const.tile([128, 3, IN_DIM], F32R)
        nc.sync.dma_start(out=w2_t[:], in_=w2.rearrange("(a p) d -> p a d", p=128))
        c2_t = const.tile([128, 3, IN_DIM], F32R)
        nc.sync.dma_start(out=c2_t[:], in_=c2.rearrange("(a p) d -> p a d", p=128))
        c1_t = const.tile([4, KDIM], F32R)
        nc.sync.dma_start(out=c1_t[:], in_=c1[:])
        boxn_t = const.tile([4, NFO], F32R)
        nc.sync.dma_start(out=boxn_t[:], in_=boxn[:])
        bcat_t = const.tile([128, 6, NFO], F32)
        nc.sync.dma_start(out=bcat_t[:], in_=bcatT.rearrange("(a p) n -> p a n", p=128))

        # ---- box MLP: bemb^T = relu(relu(boxn^T @ C1) @ C2)^T  (d-major)
        be_t = const.tile([128, 3, NFO], F32R)
        for m3 in range(3):
            pb = psA.tile([128, NFO], F32)
            nc.tensor.matmul(
                pb[:], lhsT=r(c1_t[:, bass.ts(m3, 128)]), rhs=r(boxn_t[:]),
                start=True, stop=True,
            )
            nc.scalar.activation(be_t[:, m3, :], pb[:], Relu)
        bemb_t = const.tile([128, 6, NFO], F32)
        for m6 in range(6):
            pb = psA.tile([128, NFO], F32)
            for k3 in range(3):
                nc.tensor.matmul(
                    pb[:], lhsT=r(c2_t[:, k3, bass.ts(m6, 128)]),
                    rhs=r(be_t[:, k3, :]), start=(k3 == 0), stop=(k3 == 2),
                )
            nc.scalar.activation(bemb_t[:, m6, :], pb[:], Relu)

        # ---- per-frame: A = img @ W1 ; h = relu(interp) ; o = h @ W2 ; max
        objmax_t = const.tile([128, 6, FPC, OBJ], F32)
        for f in [fr for _ in range(reps) for fr in range(FPC)]:
            imgT_t = img_pool.tile([128, 6, CELLS], F32R)
            nc.sync.dma_start(
                out=imgT_t[:], in_=imgT[f].rearrange("(a p) s -> p a s", p=128)
            )
            mall_t = mall_pool.tile([128, 2, NCOLS], F32R)
            nc.sync.dma_start(
                out=mall_t[:], in_=mall[f].rearrange("(a p) n -> p a n", p=128)
            )
            # A (s-major): psum[ms] = sum_kt imgT[kt, ms-chunk].T @ W1[kt]
            a_t = a_pool.tile([128, 2, KDIM], F32R)
            for ms in range(2):
                pa = psA.tile([128, KDIM], F32)
                for kt in range(6):
                    nc.tensor.matmul(
                        pa[:], lhsT=r(imgT_t[:, kt, bass.ts(ms, 128)]),
                        rhs=r(w1_t[:, kt, :]), start=(kt == 0), stop=(kt == 5),
                    )
                nc.scalar.copy(a_t[:, ms, :], pa[:])
            for nch in range(NCH):
                hs = []
                for m3 in range(3):
                    ph = psH.tile([128, NCHUNK], F32)
                    for st in range(2):
                        nc.tensor.matmul(
                            ph[:], lhsT=r(a_t[:, st, bass.ts(m3, 128)]),
                            rhs=r(mall_t[:, st, bass.ts(nch, NCHUNK)]),
                            start=(st == 0), stop=(st == 1),
                        )
                    h_t = h_pool.tile([128, NCHUNK], F32R)
                    nc.scalar.activation(h_t[:], ph[:], Relu)
                    hs.append(h_t)
                for m6 in range(6):
                    po = psO.tile([128, NCHUNK], F32)
                    for k3 in range(3):
                        nc.tensor.matmul(
                            po[:], lhsT=r(w2_t[:, k3, bass.ts(m6, 128)]),
                            rhs=r(hs[k3][:]), start=(k3 == 0), stop=(k3 == 2),
                        )
                    nc.vector.tensor_reduce(
                        out=objmax_t[:, m6, f, 2 * nch : 2 * nch + 2],
                        in_=po.rearrange("p (o q) -> p o q", q=CELLS),
                        axis=mybir.AxisListType.X,
                        op=mybir.AluOpType.max,
                    )

        # ---- combine: relu-fold(max) + box_emb + box_categories, d-major out
        out_t = const.tile([128, 6, NFO], F32)
        for m6 in range(6):
            nc.vector.scalar_tensor_tensor(
                out=out_t[:, m6, :],
                in0=objmax_t.rearrange("p a f o -> p a (f o)")[:, m6, :],
                scalar=0.0,
                in1=bemb_t[:, m6, :],
                op0=mybir.AluOpType.max,
                op1=mybir.AluOpType.add,
            )
            nc.vector.tensor_add(out_t[:, m6, :], out_t[:, m6, :], bcat_t[:, m6, :])
        nc.sync.dma_start(out=out.rearrange("a p n -> p a n"), in_=out_t[:])

    _split_excess_waits(nc)
    return nc


def get_module(reps=1, io_lite=False):
    key = ("nc", reps, io_lite)
    if key not in _CACHE:
        _CACHE[key] = _build_module(reps=reps, io_lite=io_lite)
    return _CACHE[key]


def make_in_maps(features, boxes, W1, W2, C1, C2, box_categories):
    features = np.asarray(features, np.float32)
    boxes = np.asarray(boxes, np.float32)
    boxes_f = boxes.reshape(BS_T, OBJ, 4)
    m_all = _build_interp_matrices(boxes_f)
    w1 = np.ascontiguousarray(np.asarray(W1, np.float32))
    w2 = np.ascontiguousarray(np.asarray(W2, np.float32))
    c1 = np.ascontiguousarray(np.asarray(C1, np.float32))
    c2 = np.ascontiguousarray(np.asarray(C2, np.float32))
    bcat = np.asarray(box_categories, np.float32)

    in_maps = []
    for c in range(N_CORES):
        fr = slice(c * FPC, (c + 1) * FPC)
        imgT = np.ascontiguousarray(
            features[1:, fr, :].transpose(1, 2, 0)  # (6, 768, 256)
        )
        mall_c = np.ascontiguousarray(m_all[fr])
        boxn = np.ascontiguousarray(
            (boxes_f[fr].reshape(FPC * OBJ, 4).T / np.float32(VIDEO_HW))
        )
        t_idx = (np.arange(c * FPC, (c + 1) * FPC)) % NB_FRAMES
        bcatT = np.ascontiguousarray(
            bcat[t_idx].reshape(FPC * OBJ, IN_DIM).T  # (768, 60)
        )
        in_maps.append(
            {
                "imgT": imgT,
                "mall": mall_c,
                "w1": w1,
                "w2": w2,
                "c1": c1,
                "c2": c2,
                "boxn": boxn,
                "bcatT": bcatT,
            }
        )
    return in_maps


def assemble_output(results, boxes):
    boxes_f = np.asarray(boxes, np.float32).reshape(BS_T, OBJ, 4)
    object_tokens = np.empty((OBJ, BS_T, IN_DIM), np.float32)
    for c in range(N_CORES):
        arr = results[c]["out"].reshape(IN_DIM, FPC, OBJ)  # (768, 6, 10)
        object_tokens[:, c * FPC : (c + 1) * FPC, :] = arr.transpose(2, 1, 0)
    attn_mask = boxes_f[:, :, 0] == -1.0
    return object_tokens, attn_mask


def kernel(features, boxes, W1, W2, C1, C2, box_categories):
    nc = get_module()
    in_maps = make_in_maps(features, boxes, W1, W2, C1, C2, box_categories)
    res = run_bass_kernel_spmd(nc, in_maps, list(range(N_CORES)))
    return assemble_output(res.results, boxes)


# revision 12
# speedup vs baseline: 997.3478x; 11.2753x over previous
"""Trainium2 Bass kernel for the ObjectEncoder problem.

Pipeline (data-parallel over bs_t=48 frames, 6 frames per core x 8 cores):
  host: build per-(frame,box) separable bilinear-interp matrices and fuse
        them into one dense kron matrix M_all[s=(y,x), (o,p,q)] per frame.
        roi_align is linear in the image, so
           roi_align(img) @ W1 == M_all.T @ (img @ W1)
        which shrinks the interp matmul from d=768 to k=384 columns.
  core: A   = img @ W1                      (256, 384)  per frame
        h^T = relu(A^T-by-chunks @ M_all)   (384, 2560) per frame
        o^T = h^T.T-contract @ W2           (768, 2560) -> max over 256 cells
        out = relu-fold(max) + box_mlp(boxes) + box_categories
Matmuls run in bf16 (fp32/fp32r stream at ~1/4 rate on this PE).
"""

import numpy as np
from contextlib import ExitStack

import concourse.bass as bass
import concourse.mybir as mybir
from concourse import tile
from concourse.bass_utils import run_bass_kernel_spmd

F32 = mybir.dt.float32
F32R = mybir.dt.float32r
BF16 = mybir.dt.bfloat16
MM_DTYPE = BF16  # matmul operand dtype for the default module

N_CORES = 8
PATCH = 16
IN_DIM = 768
KDIM = 384
NB_FRAMES = 12
OBJ = 10
BS = 4
BS_T = BS * NB_FRAMES            # 48
FPC = BS_T // N_CORES            # 6 frames per core
CELLS = PATCH * PATCH            # 256
NCOLS = OBJ * CELLS              # 2560
NCHUNK = 512                     # psum-bank-sized moving chunks
NCH = NCOLS // NCHUNK            # 5
VIDEO_HW = 224.0


def _patch_tile_drain():
    """walrus in this container allows only 1 sync-wait on a CTRL (Drain)
    instruction; spread the Tile tail-drain waits across SP nops."""
    if getattr(tile.TileContext, "_drain_patched", False):
        return
    ScopedClock = tile.ScopedClock

    def _drain_and_barrier(self, tick_clock, wait_clock):
        probe = self.nc.sync.drain()
        wait_clock.add_sem_waits(
            probe.ins, ScopedClock({None: tick_clock.global_clock})
        )
        waits = list(probe.ins.sync_info.on_wait)
        probe.ins.sync_info.on_wait = waits[:1]
        for w in waits[1:]:
            nop = self.nc.sync.nop(nofuse=True)
            nop.ins.sync_info = mybir.SyncInfo(on_wait=[w], on_update=[])
        self.nc.all_engine_barrier()
        popped = self.nc._tile_sem_poison_stack.pop()
        assert popped is self._sem_poison
        self.nc.clear_and_free_semaphores(list(self.sems.allocated().values()))
        self.nc.all_engine_barrier()

    tile.TileContext._drain_and_barrier = _drain_and_barrier
    tile.TileContext._drain_patched = True


_WAIT_CAP = 1
_SPLIT_SKIP = set()


def _split_excess_waits(nc):
    """This container's walrus rejects >1 sync-wait on compute/CTRL
    instructions; move excess waits onto same-engine NoOps placed before."""
    ctr = [0]
    for fn in nc.m.functions:
        for bb in fn.blocks:
            new_list = []
            for inst in bb.instructions:
                si = getattr(inst, "sync_info", None)
                waits = list(si.on_wait) if si and si.on_wait else []
                if (
                    len(waits) > _WAIT_CAP
                    and type(inst).__name__ not in _SPLIT_SKIP
                ):
                    for w in waits[:-_WAIT_CAP]:
                        ctr[0] += 1
                        nop = mybir.InstNoOp(
                            name=f"WSPLIT-{ctr[0]}", ins=[], outs=[]
                        )
                        nop.engine = inst.engine
                        nop.sync_info = mybir.SyncInfo(
                            on_wait=[w], on_update=[]
                        )
                        new_list.append(nop)
                    si.on_wait = waits[-_WAIT_CAP:]
                new_list.append(inst)
            bb.instructions = new_list


def _axis_interp_np(coord, size):
    # mirrors reference._axis_interp (torchvision bilinear semantics)
    valid = ((coord >= -1.0) & (coord <= size)).astype(coord.dtype)
    c = np.maximum(coord, np.float32(0.0))
    i0 = np.minimum(np.floor(c).astype(np.int64), size - 1)
    i1 = np.minimum(i0 + 1, size - 1)
    l = np.clip(c - i0.astype(coord.dtype), 0.0, 1.0)
    w0 = (1.0 - l) * valid
    w1 = l * valid
    return i0, i1, w0, w1


def _build_interp_matrices(boxes_f):
    """boxes_f: (F, O, 4) fp32 -> M_all (F, 256, O*256) fp32 with
    M_all[f, y*16+x, o*256+p*16+q] = My[f,o,p,y] * Mx[f,o,q,x]."""
    F = boxes_f.shape[0]
    scale = np.float32(PATCH / VIDEO_HW)
    b = boxes_f.astype(np.float32) * scale - np.float32(0.5)
    x1, y1, x2, y2 = b[..., 0], b[..., 1], b[..., 2], b[..., 3]
    grid = (np.arange(PATCH, dtype=np.float32) + np.float32(0.5))
    ys = y1[..., None] + grid * ((y2 - y1) / PATCH)[..., None]   # (F,O,P)
    xs = x1[..., None] + grid * ((x2 - x1) / PATCH)[..., None]   # (F,O,P)

    def weight_mat(coord):
        i0, i1, w0, w1 = _axis_interp_np(coord, PATCH)
        M = np.zeros((F, OBJ, PATCH, PATCH), np.float32)
        fi, oi, pi = np.indices(i0.shape)
        np.add.at(M, (fi, oi, pi, i0), w0)
        np.add.at(M, (fi, oi, pi, i1), w1)
        return M

    My = weight_mat(ys)   # (F, O, P, 16) over y
    Mx = weight_mat(xs)   # (F, O, P, 16) over x
    M_all = np.einsum("fopy,foqx->fyxopq", My, Mx)
    return np.ascontiguousarray(M_all.reshape(F, CELLS, NCOLS))


_CACHE = {}


def _build_module(reps=1, io_lite=False, mmdt=None):
    """reps>1 replicates the whole frame pipeline for wall-clock timing
    amplification; io_lite turns the big inputs into internal (uninitialized)
    DRAM so timing runs don't pay host->device transfer."""
    if mmdt is None:
        mmdt = MM_DTYPE
    _patch_tile_drain()
    nc = bass.Bass()
    if io_lite:
        imgT = nc.dram_tensor("imgT_i", [FPC, IN_DIM, CELLS], mmdt)
        mall = nc.dram_tensor("mall_i", [FPC, CELLS, NCOLS], mmdt)
    else:
        imgT = nc.declare_dram_parameter(
            "imgT", [FPC, IN_DIM, CELLS], mmdt, isOutput=False
        )
        mall = nc.declare_dram_parameter(
            "mall", [FPC, CELLS, NCOLS], mmdt, isOutput=False
        )
    w1 = nc.declare_dram_parameter("w1", [IN_DIM, KDIM], mmdt, isOutput=False)
    w2 = nc.declare_dram_parameter("w2", [KDIM, IN_DIM], mmdt, isOutput=False)
    c1 = nc.declare_dram_parameter("c1", [4, KDIM], mmdt, isOutput=False)
    c2 = nc.declare_dram_parameter("c2", [KDIM, IN_DIM], mmdt, isOutput=False)
    boxn = nc.declare_dram_parameter("boxn", [4, FPC * OBJ], mmdt, isOutput=False)
    bcatT = nc.declare_dram_parameter("bcatT", [IN_DIM, FPC * OBJ], F32, isOutput=False)
    out = nc.declare_dram_parameter("out", [6, 128, FPC * OBJ], F32, isOutput=True)

    r = lambda ap: ap
    Relu = mybir.ActivationFunctionType.Relu
    NFO = FPC * OBJ  # 60

    tc = tile.TileContext(nc)
    with tc, ExitStack() as ctx:
        const = ctx.enter_context(tc.tile_pool(name="const", bufs=1))
        img_pool = ctx.enter_context(tc.tile_pool(name="img", bufs=2))
        mall_pool = ctx.enter_context(tc.tile_pool(name="mall", bufs=2))
        a_pool = ctx.enter_context(tc.tile_pool(name="a", bufs=2))
        h_pool = ctx.enter_context(tc.tile_pool(name="h", bufs=6))
        psA = ctx.enter_context(tc.tile_pool(name="psA", bufs=2, space="PSUM"))
        psH = ctx.enter_context(tc.tile_pool(name="psH", bufs=2, space="PSUM"))
        psO = ctx.enter_context(tc.tile_pool(name="psO", bufs=2, space="PSUM"))

        # ---- constants
        w1_t = const.tile([128, 6, KDIM], mmdt)
        nc.sync.dma_start(out=w1_t[:], in_=w1.rearrange("(a p) k -> p a k", p=128))
        w2_t = const.tile([128, 3, IN_DIM], mmdt)
        nc.sync.dma_start(out=w2_t[:], in_=w2.rearrange("(a p) d -> p a d", p=128))
        c2_t = const.tile([128, 3, IN_DIM], mmdt)
        nc.sync.dma_start(out=c2_t[:], in_=c2.rearrange("(a p) d -> p a d", p=128))
        c1_t = const.tile([4, KDIM], mmdt)
        nc.sync.dma_start(out=c1_t[:], in_=c1[:])
        boxn_t = const.tile([4, NFO], mmdt)
        nc.sync.dma_start(out=boxn_t[:], in_=boxn[:])
        bcat_t = const.tile([128, 6, NFO], F32)
        nc.sync.dma_start(out=bcat_t[:], in_=bcatT.rearrange("(a p) n -> p a n", p=128))

        # ---- box MLP: bemb^T = relu(relu(boxn^T @ C1) @ C2)^T  (d-major)
        be_t = const.tile([128, 3, NFO], mmdt)
        for m3 in range(3):
            pb = psA.tile([128, NFO], F32)
            nc.tensor.matmul(
                pb[:], lhsT=r(c1_t[:, bass.ts(m3, 128)]), rhs=r(boxn_t[:]),
                start=True, stop=True,
            )
            nc.scalar.activation(be_t[:, m3, :], pb[:], Relu)
        bemb_t = const.tile([128, 6, NFO], F32)
        for m6 in range(6):
            pb = psA.tile([128, NFO], F32)
            for k3 in range(3):
                nc.tensor.matmul(
                    pb[:], lhsT=r(c2_t[:, k3, bass.ts(m6, 128)]),
                    rhs=r(be_t[:, k3, :]), start=(k3 == 0), stop=(k3 == 2),
                )
            nc.scalar.activation(bemb_t[:, m6, :], pb[:], Relu)

        # ---- per-frame: A = img @ W1 ; h = relu(interp) ; o = h @ W2 ; max
        objmax_t = const.tile([128, 6, FPC, OBJ], F32)
        for f in [fr for _ in range(reps) for fr in range(FPC)]:
            imgT_t = img_pool.tile([128, 6, CELLS], mmdt)
            nc.sync.dma_start(
                out=imgT_t[:], in_=imgT[f].rearrange("(a p) s -> p a s", p=128)
            )
            mall_t = mall_pool.tile([128, 2, NCOLS], mmdt)
            nc.sync.dma_start(
                out=mall_t[:], in_=mall[f].rearrange("(a p) n -> p a n", p=128)
            )
            # A (s-major): psum[ms] = sum_kt imgT[kt, ms-chunk].T @ W1[kt]
            a_t = a_pool.tile([128, 2, KDIM], mmdt)
            for ms in range(2):
                pa = psA.tile([128, KDIM], F32)
                for kt in range(6):
                    nc.tensor.matmul(
                        pa[:], lhsT=r(imgT_t[:, kt, bass.ts(ms, 128)]),
                        rhs=r(w1_t[:, kt, :]), start=(kt == 0), stop=(kt == 5),
                    )
                nc.scalar.copy(a_t[:, ms, :], pa[:])
            for nch in range(NCH):
                hs = []
                for m3 in range(3):
                    ph = psH.tile([128, NCHUNK], F32)
                    for st in range(2):
                        nc.tensor.matmul(
                            ph[:], lhsT=r(a_t[:, st, bass.ts(m3, 128)]),
                            rhs=r(mall_t[:, st, bass.ts(nch, NCHUNK)]),
                            start=(st == 0), stop=(st == 1),
                        )
                    h_t = h_pool.tile([128, NCHUNK], mmdt)
                    nc.scalar.activation(h_t[:], ph[:], Relu)
                    hs.append(h_t)
                for m6 in range(6):
                    po = psO.tile([128, NCHUNK], F32)
                    for k3 in range(3):
                        nc.tensor.matmul(
                            po[:], lhsT=r(w2_t[:, k3, bass.ts(m6, 128)]),
                            rhs=r(hs[k3][:]), start=(k3 == 0), stop=(k3 == 2),
                        )
                    nc.vector.tensor_reduce(
                        out=objmax_t[:, m6, f, 2 * nch : 2 * nch + 2],
                        in_=po.rearrange("p (o q) -> p o q", q=CELLS),
                        axis=mybir.AxisListType.X,
                        op=mybir.AluOpType.max,
                    )

        # ---- combine: relu-fold(max) + box_emb + box_categories, d-major out
        out_t = const.tile([128, 6, NFO], F32)
        for m6 in range(6):
            nc.vector.scalar_tensor_tensor(
                out=out_t[:, m6, :],
                in0=objmax_t.rearrange("p a f o -> p a (f o)")[:, m6, :],
                scalar=0.0,
                in1=bemb_t[:, m6, :],
                op0=mybir.AluOpType.max,
                op1=mybir.AluOpType.add,
            )
            nc.vector.tensor_add(out_t[:, m6, :], out_t[:, m6, :], bcat_t[:, m6, :])
        nc.sync.dma_start(out=out.rearrange("a p n -> p a n"), in_=out_t[:])

    _split_excess_waits(nc)
    return nc


def get_module(reps=1, io_lite=False, mmdt=None):
    if mmdt is None:
        mmdt = MM_DTYPE
    key = ("nc", reps, io_lite, str(mmdt))
    if key not in _CACHE:
        _CACHE[key] = _build_module(reps=reps, io_lite=io_lite, mmdt=mmdt)
    return _CACHE[key]


def _mm_np_dtype():
    import ml_dtypes

    return ml_dtypes.bfloat16 if MM_DTYPE == BF16 else np.float32


def make_in_maps(features, boxes, W1, W2, C1, C2, box_categories):
    mdt = _mm_np_dtype()
    features = np.asarray(features, np.float32)
    boxes = np.asarray(boxes, np.float32)
    boxes_f = boxes.reshape(BS_T, OBJ, 4)
    m_all = _build_interp_matrices(boxes_f)
    w1 = np.ascontiguousarray(np.asarray(W1, np.float32))
    w2 = np.ascontiguousarray(np.asarray(W2, np.float32))
    c1 = np.ascontiguousarray(np.asarray(C1, np.float32))
    c2 = np.ascontiguousarray(np.asarray(C2, np.float32))
    bcat = np.asarray(box_categories, np.float32)

    in_maps = []
    for c in range(N_CORES):
        fr = slice(c * FPC, (c + 1) * FPC)
        imgT = np.ascontiguousarray(
            features[1:, fr, :].transpose(1, 2, 0)  # (6, 768, 256)
        )
        mall_c = np.ascontiguousarray(m_all[fr])
        boxn = np.ascontiguousarray(
            (boxes_f[fr].reshape(FPC * OBJ, 4).T / np.float32(VIDEO_HW))
        )
        t_idx = (np.arange(c * FPC, (c + 1) * FPC)) % NB_FRAMES
        bcatT = np.ascontiguousarray(
            bcat[t_idx].reshape(FPC * OBJ, IN_DIM).T  # (768, 60)
        )
        in_maps.append(
            {
                "imgT": imgT.astype(mdt),
                "mall": mall_c.astype(mdt),
                "w1": w1.astype(mdt),
                "w2": w2.astype(mdt),
                "c1": c1.astype(mdt),
                "c2": c2.astype(mdt),
                "boxn": boxn.astype(mdt),
                "bcatT": bcatT,
            }
        )
    return in_maps


def assemble_output(results, boxes):
    boxes_f = np.asarray(boxes, np.float32).reshape(BS_T, OBJ, 4)
    object_tokens = np.empty((OBJ, BS_T, IN_DIM), np.float32)
    for c in range(N_CORES):
        arr = results[c]["out"].reshape(IN_DIM, FPC, OBJ)  # (768, 6, 10)
        object_tokens[:, c * FPC : (c + 1) * FPC, :] = arr.transpose(2, 1, 0)
    attn_mask = boxes_f[:, :, 0] == -1.0
    return object_tokens, attn_mask


def kernel(features, boxes, W1, W2, C1, C2, box_categories):
    nc = get_module()
    in_maps = make_in_maps(features, boxes, W1, W2, C1, C2, box_categories)
    res = run_bass_kernel_spmd(nc, in_maps, list(range(N_CORES)))
    return assemble_output(res.results, boxes)
